# revision 25
# baseline (speedup 1.0000x reference)
"""Grouped-experts SwiGLU FFN (MoE) on 8 Trainium2 NeuronCores.

Expert-parallel: core e owns expert e's weights and its contiguous token
slice.  Tokens are already sorted by expert (contiguous ranges from
cumsum(num_tokens_per_expert)), so the "all-to-all dispatch" is plain host
slicing.  Each core runs a two-stage SwiGLU:

  stage 1:  HT[h, t] = silu(W1 x)[h, t] * (W3 x)[h, t]     (K = DIM)
  stage 2:  OUT.T[d, t] = (W2 @ H)[d, t]                   (K = HIDDEN)

Matmuls run in bf16 (1 cycle/row on the PE array, fp32 PSUM accumulate);
~4e-3 relative error vs the fp32 reference.  fp8 DoubleRow (2x FLOPs) was
measured on hardware but plain-fp8 quantization costs 4-7e-2 rel err
(gate is 2e-2) and hi/lo-split fp8 needs 3 passes = 1.5x bf16 time, so
bf16 is optimal here.  The kernel is PE-bound (roofline ~451us/core at
2.4GHz); the schedule keeps the PE saturated from program start to end:

  - Stage 1 runs token-chunk-outer so all w1/w3 blocks stay resident in
    SBUF; every weight DMA issues immediately and the PE has no weight
    waits after the first chunk.  The w1 and w3 chains are interleaved
    per contraction block so each x quarter feeds both chains the moment
    it lands, halving early DMA demand per PE-second.
  - Each HW DGE queue transfers serially, so the startup stream on Sync
    is ordered exactly by first use (w1[0]q/x0q/w3[0]q quarter triplets,
    then remaining weights); Scalar carries the later x chunks
    (prefetched one group ahead) and the stage-2 output stores.
  - A dummy-matmul train (memset tile) keeps the PE busy from ~7.5us to
    the first operands (~15us) so the HAM clock ramp and its half-speed
    window complete before real work (early DMA is the wall: ~2.5MB by
    ~16us from program start).
  - Outputs are stored as bf16 (host upcasts), halving store traffic;
    the final chunk runs as two half-chains to shorten the tail drain.
"""

import numpy as np
import ml_dtypes

import concourse.bass as bass
from concourse import bacc
import concourse.mybir as mybir
from concourse.tile import TileContext
from concourse.bass_utils import run_bass_kernel_spmd

N_TOKENS = 16384
DIM = 2048
HIDDEN = 1408
N_EXPERTS = 8
N_CORES = 8

P = 128
T = 2048                 # token capacity per core per pass
N_DN = DIM // P          # 16 contraction blocks in stage 1
N_HT = HIDDEN // P       # 11 h tiles
N_DT = DIM // P          # 16 output-row tiles in stage 2
TSUB = 512               # moving-operand width per matmul (1 PSUM bank)
N_DUMMY = 32             # PE warm-up matmuls
DUMMY_ROWS = 384         # moving rows per warm-up matmul

F32 = mybir.dt.float32
BF16 = mybir.dt.bfloat16
SILU = mybir.ActivationFunctionType.Silu
BF = ml_dtypes.bfloat16


def _build_program() -> bass.Bass:
    nc = bacc.Bacc()
    xtp = nc.declare_dram_parameter(
        "xtp", [P, T // TSUB, N_DN, TSUB], BF16, isOutput=False)
    w1p = nc.declare_dram_parameter("w1p", [P, N_HT, N_DN, P], BF16, isOutput=False)
    w3p = nc.declare_dram_parameter("w3p", [P, N_HT, N_DN, P], BF16, isOutput=False)
    w2p = nc.declare_dram_parameter("w2p", [P, N_DT, N_HT, P], BF16, isOutput=False)
    outt = nc.declare_dram_parameter("outt", [DIM, T], BF16, isOutput=True)

    with TileContext(nc) as tc:
        with (
            tc.tile_pool(name="xt", bufs=2) as xt_pool,
            tc.tile_pool(name="ht", bufs=1) as ht_pool,
            tc.tile_pool(name="w1", bufs=1) as w1_pool,
            tc.tile_pool(name="w3", bufs=1) as w3_pool,
            tc.tile_pool(name="w2", bufs=3) as w2_pool,
            tc.tile_pool(name="tmp", bufs=2) as tmp_pool,
            tc.tile_pool(name="ob", bufs=2) as ob_pool,
            tc.tile_pool(name="dmy", bufs=1) as dmy_pool,
            tc.tile_pool(name="ps1", bufs=3, space="PSUM") as ps1_pool,
            tc.tile_pool(name="ps2", bufs=3, space="PSUM") as ps2_pool,
            tc.tile_pool(name="pso", bufs=2, space="PSUM") as pso_pool,
        ):
            # PE pre-warm: dep-free dummy matmuls keep the PE busy (HAM
            # clock ramp) while the first operands stream in.  memset on
            # the vector engine - it exits the boot barrier early.
            dmy = dmy_pool.tile([P, TSUB], BF16)
            nc.vector.memset(dmy[:], 0.0)
            for i in range(N_DUMMY):
                psd = pso_pool.tile([P, TSUB], F32, name="pso")
                nc.tensor.matmul(psd[:, 0:DUMMY_ROWS],
                                 lhsT=dmy[:, 0:P], rhs=dmy[:, 0:DUMMY_ROWS])

            # All w1/w3 weight blocks stay resident in SBUF for the whole of
            # stage 1 (token chunks are the outer loop), so every weight DMA
            # can issue immediately and the PE runs gap-free once the first
            # chunk has landed.
            w1bs = [w1_pool.tile([P, N_DN, P], BF16, name=f"w1_{ih}")
                    for ih in range(N_HT)]
            w3bs = [w3_pool.tile([P, N_DN, P], BF16, name=f"w3_{ih}")
                    for ih in range(N_HT)]
            xc0 = xt_pool.tile([P, N_DN, TSUB], BF16, name="xc")
            # Startup loads in exact first-use order, quarter granularity.
            # Each HW queue transfers serially: the first chain's operand
            # stream (w1[0]/x0 quarter pairs, then the remaining weights) on
            # Sync; w3[0] + the later x chunks on Scalar in parallel.
            for q in range(4):
                dn = slice(q * 4, (q + 1) * 4)
                nc.sync.dma_start(out=w1bs[0][:, dn, :], in_=w1p[:, 0, dn, :])
                nc.sync.dma_start(out=xc0[:, dn, :], in_=xtp[:, 0, dn, :])
                nc.sync.dma_start(out=w3bs[0][:, dn, :], in_=w3p[:, 0, dn, :])
            for ih in range(1, N_HT):
                nc.sync.dma_start(out=w1bs[ih][:], in_=w1p[:, ih, :, :])
                nc.sync.dma_start(out=w3bs[ih][:], in_=w3p[:, ih, :, :])
            ht = ht_pool.tile([P, N_HT, T], BF16)

            # stage 1: HT[h, t] = silu(x @ w1.T).T * (x @ w3.T).T
            # token chunk its+1 is prefetched at the top of group its so the
            # group boundary never waits on DMA (xt pool double-buffers)
            xts = [xc0] + [None] * (T // TSUB - 1)
            for its in range(T // TSUB):
                ts0 = its * TSUB
                if its + 1 < T // TSUB:
                    nxt = xt_pool.tile([P, N_DN, TSUB], BF16, name="xc")
                    nc.scalar.dma_start(out=nxt[:], in_=xtp[:, its + 1, :, :])
                    xts[its + 1] = nxt
                xt_c = xts[its]
                for ih in range(N_HT):
                    ps1 = ps1_pool.tile([P, TSUB], F32)
                    ps2 = ps2_pool.tile([P, TSUB], F32)
                    # w1 and w3 chains interleaved per n-block: each x
                    # quarter is consumed by both chains as soon as it
                    # lands, halving the early DMA demand per PE-second
                    for n in range(N_DN):
                        nc.tensor.matmul(
                            ps1[:],
                            lhsT=w1bs[ih][:, n, :],
                            rhs=xt_c[:, n, :],
                            start=(n == 0),
                            stop=(n == N_DN - 1),
                        )
                        nc.tensor.matmul(
                            ps2[:],
                            lhsT=w3bs[ih][:, n, :],
                            rhs=xt_c[:, n, :],
                            start=(n == 0),
                            stop=(n == N_DN - 1),
                        )
                    tmp = tmp_pool.tile([P, TSUB], F32)
                    nc.scalar.activation(tmp[:], ps1[:], SILU)
                    nc.vector.tensor_mul(
                        ht[:, ih, ts0:ts0 + TSUB], tmp[:], ps2[:]
                    )

            # stage 2: OUT.T[d, t] = sum_h W2T[h, d] * HT[h, t]
            for idt in range(N_DT):
                w2b = w2_pool.tile([P, N_HT, P], BF16)
                nc.sync.dma_start(out=w2b[:], in_=w2p[:, idt, :, :])
                for its in range(T // TSUB):
                    ts0 = its * TSUB
                    last = (idt == N_DT - 1) and (its == T // TSUB - 1)
                    # the very last chunk runs as two half-width chains so
                    # the copy+store of the first half hides behind the
                    # second half's matmuls; the final store issues on the
                    # idle Sync queue, shortening the kernel tail
                    pieces = ([(0, 256, nc.scalar), (256, 256, nc.sync)]
                              if last else [(0, TSUB, nc.scalar)])
                    for h0, hw, eng in pieces:
                        pso = pso_pool.tile([P, TSUB], F32, name="pso")
                        for hn in range(N_HT):
                            nc.tensor.matmul(
                                pso[:, 0:hw],
                                lhsT=w2b[:, hn, :],
                                rhs=ht[:, hn, ts0 + h0:ts0 + h0 + hw],
                                start=(hn == 0),
                                stop=(hn == N_HT - 1),
                            )
                        ob = ob_pool.tile([P, TSUB], BF16, name="ob")
                        nc.vector.tensor_copy(ob[:, 0:hw], pso[:, 0:hw])
                        eng.dma_start(
                            out=outt[idt * P:(idt + 1) * P,
                                     ts0 + h0:ts0 + h0 + hw],
                            in_=ob[:, 0:hw],
                        )
    nc.compile()
    return nc


_CACHE: dict = {}


def _get_nc() -> bass.Bass:
    if "nc" not in _CACHE:
        _CACHE["nc"] = _build_program()
    return _CACHE["nc"]


def _pack_weights(w1, w2, w3):
    maps = []
    for e in range(N_EXPERTS):
        maps.append({
            "w1p": np.ascontiguousarray(
                w1[e].reshape(N_HT, P, N_DN, P).transpose(3, 0, 2, 1).astype(BF)),
            "w3p": np.ascontiguousarray(
                w3[e].reshape(N_HT, P, N_DN, P).transpose(3, 0, 2, 1).astype(BF)),
            "w2p": np.ascontiguousarray(
                w2[e].reshape(N_DT, P, N_HT, P).transpose(3, 0, 2, 1).astype(BF)),
        })
    return maps


def kernel(x, w1, w2, w3, num_tokens_per_expert, _trace=False):
    x = np.ascontiguousarray(np.asarray(x, dtype=np.float32))
    w1 = np.ascontiguousarray(np.asarray(w1, dtype=np.float32))
    w2 = np.ascontiguousarray(np.asarray(w2, dtype=np.float32))
    w3 = np.ascontiguousarray(np.asarray(w3, dtype=np.float32))
    counts = np.asarray(num_tokens_per_expert, dtype=np.int64)

    cs = np.cumsum(counts)
    starts = np.minimum(np.concatenate([[0], cs[:-1]]), N_TOKENS)
    ends = np.minimum(cs, N_TOKENS)
    lens = np.maximum(ends - starts, 0)

    wmaps = _pack_weights(w1, w2, w3)
    out = np.zeros((N_TOKENS, DIM), np.float32)
    trace_info = []

    n_passes = max(1, int(np.max(np.ceil(lens / T))))
    for k in range(n_passes):
        in_maps = []
        for e in range(N_EXPERTS):
            s = int(starts[e]) + k * T
            xe = np.zeros((T, DIM), np.float32)
            avail = x[s:s + T]
            if avail.shape[0]:
                xe[:avail.shape[0]] = avail
            # [P, n_chunks, N_DN, TSUB]: xtp[p, c, n, t] = x[c*TSUB+t, n*128+p]
            xtp = np.ascontiguousarray(
                xe.T.reshape(N_DN, P, T // TSUB, TSUB)
                .transpose(1, 2, 0, 3).astype(BF))
            in_maps.append({"xtp": xtp, **wmaps[e]})
        res = run_bass_kernel_spmd(
            _get_nc(), in_maps, list(range(N_CORES)), trace=_trace
        )
        if _trace:
            trace_info.append(res)
        for e in range(N_EXPERTS):
            s = int(starts[e]) + k * T
            cnt = min(int(ends[e]) - s, T)
            if cnt > 0:
                out[s:s + cnt] = res.results[e]["outt"].T[:cnt].astype(np.float32)

    if _trace:
        return out, trace_info
    return out


# revision 26
# speedup vs baseline: 1.0120x; 1.0120x over previous
"""Grouped-experts SwiGLU FFN (MoE) on 8 Trainium2 NeuronCores.

Expert-parallel: core e owns expert e's weights and its contiguous token
slice.  Tokens are already sorted by expert (contiguous ranges from
cumsum(num_tokens_per_expert)), so the "all-to-all dispatch" is plain host
slicing.  Each core runs a two-stage SwiGLU:

  stage 1:  HT[h, t] = silu(W1 x)[h, t] * (W3 x)[h, t]     (K = DIM)
  stage 2:  OUT.T[d, t] = (W2 @ H)[d, t]                   (K = HIDDEN)

Matmuls run in bf16 (1 cycle/row on the PE array, fp32 PSUM accumulate);
~4e-3 relative error vs the fp32 reference.  fp8 DoubleRow (2x FLOPs) was
measured on hardware but plain-fp8 quantization costs 4-7e-2 rel err
(gate is 2e-2) and hi/lo-split fp8 needs 3 passes = 1.5x bf16 time, so
bf16 is optimal here.  The kernel is PE-bound (roofline ~451us/core at
2.4GHz); the schedule keeps the PE saturated from program start to end:

  - Stage 1 runs token-chunk-outer so all w1/w3 blocks stay resident in
    SBUF; every weight DMA issues immediately and the PE has no weight
    waits after the first chunk.  The w1 and w3 chains are interleaved
    per contraction block so each x quarter feeds both chains the moment
    it lands, halving early DMA demand per PE-second.
  - Each HW DGE queue transfers serially, so the startup stream on Sync
    is ordered exactly by first use (w1[0]q/x0q/w3[0]q quarter triplets,
    then remaining weights); Scalar carries the later x chunks
    (prefetched one group ahead) and the stage-2 output stores.
  - A dummy-matmul train (memset tile) keeps the PE busy from ~7.5us to
    the first operands (~15us) so the HAM clock ramp and its half-speed
    window complete before real work (early DMA is the wall: ~2.5MB by
    ~16us from program start).
  - Outputs are stored as bf16 (host upcasts), halving store traffic;
    the final chunk runs as two half-chains to shorten the tail drain.
"""

import numpy as np
import ml_dtypes

import concourse.bass as bass
from concourse import bacc
import concourse.mybir as mybir
from concourse.tile import TileContext
from concourse.bass_utils import run_bass_kernel_spmd

N_TOKENS = 16384
DIM = 2048
HIDDEN = 1408
N_EXPERTS = 8
N_CORES = 8

P = 128
T = 2048                 # token capacity per core per pass
N_DN = DIM // P          # 16 contraction blocks in stage 1
N_HT = HIDDEN // P       # 11 h tiles
N_DT = DIM // P          # 16 output-row tiles in stage 2
TSUB = 512               # moving-operand width per matmul (1 PSUM bank)
N_DUMMY = 40             # PE warm-up matmuls
DUMMY_ROWS = 384         # moving rows per warm-up matmul

F32 = mybir.dt.float32
BF16 = mybir.dt.bfloat16
SILU = mybir.ActivationFunctionType.Silu
BF = ml_dtypes.bfloat16


def _build_program() -> bass.Bass:
    nc = bacc.Bacc()
    xtp = nc.declare_dram_parameter(
        "xtp", [P, T // TSUB, N_DN, TSUB], BF16, isOutput=False)
    w1p = nc.declare_dram_parameter("w1p", [P, N_HT, N_DN, P], BF16, isOutput=False)
    w3p = nc.declare_dram_parameter("w3p", [P, N_HT, N_DN, P], BF16, isOutput=False)
    w2p = nc.declare_dram_parameter("w2p", [P, N_DT, N_HT, P], BF16, isOutput=False)
    outt = nc.declare_dram_parameter("outt", [DIM, T], BF16, isOutput=True)

    with TileContext(nc) as tc:
        with (
            tc.tile_pool(name="xt", bufs=2) as xt_pool,
            tc.tile_pool(name="ht", bufs=1) as ht_pool,
            tc.tile_pool(name="w1", bufs=1) as w1_pool,
            tc.tile_pool(name="w3", bufs=1) as w3_pool,
            tc.tile_pool(name="w2", bufs=3) as w2_pool,
            tc.tile_pool(name="tmp", bufs=2) as tmp_pool,
            tc.tile_pool(name="ob", bufs=2) as ob_pool,
            tc.tile_pool(name="dmy", bufs=1) as dmy_pool,
            tc.tile_pool(name="ps1", bufs=3, space="PSUM") as ps1_pool,
            tc.tile_pool(name="ps2", bufs=3, space="PSUM") as ps2_pool,
            tc.tile_pool(name="pso", bufs=2, space="PSUM") as pso_pool,
        ):
            # PE pre-warm: dep-free dummy matmuls keep the PE busy (HAM
            # clock ramp) while the first operands stream in.  memset on
            # the vector engine - it exits the boot barrier early.
            dmy = dmy_pool.tile([P, TSUB], BF16)
            nc.vector.memset(dmy[:], 0.0)
            for i in range(N_DUMMY):
                psd = pso_pool.tile([P, TSUB], F32, name="pso")
                nc.tensor.matmul(psd[:, 0:DUMMY_ROWS],
                                 lhsT=dmy[:, 0:P], rhs=dmy[:, 0:DUMMY_ROWS])

            # All w1/w3 weight blocks stay resident in SBUF for the whole of
            # stage 1 (token chunks are the outer loop), so every weight DMA
            # can issue immediately and the PE runs gap-free once the first
            # chunk has landed.
            w1bs = [w1_pool.tile([P, N_DN, P], BF16, name=f"w1_{ih}")
                    for ih in range(N_HT)]
            w3bs = [w3_pool.tile([P, N_DN, P], BF16, name=f"w3_{ih}")
                    for ih in range(N_HT)]
            xc0 = xt_pool.tile([P, N_DN, TSUB], BF16, name="xc")
            # Startup loads in exact first-use order, quarter granularity.
            # Each HW queue transfers serially: the first chain's operand
            # stream (w1[0]/x0 quarter pairs, then the remaining weights) on
            # Sync; w3[0] + the later x chunks on Scalar in parallel.
            for q in range(4):
                dn = slice(q * 4, (q + 1) * 4)
                nc.sync.dma_start(out=w1bs[0][:, dn, :], in_=w1p[:, 0, dn, :])
                nc.sync.dma_start(out=xc0[:, dn, :], in_=xtp[:, 0, dn, :])
                nc.sync.dma_start(out=w3bs[0][:, dn, :], in_=w3p[:, 0, dn, :])
            # ih=1 weights quartered right behind the startup triplets so
            # the second chain-pair never waits; later ih stream whole
            for q in range(4):
                dn = slice(q * 4, (q + 1) * 4)
                nc.sync.dma_start(out=w1bs[1][:, dn, :], in_=w1p[:, 1, dn, :])
                nc.sync.dma_start(out=w3bs[1][:, dn, :], in_=w3p[:, 1, dn, :])
            for ih in range(2, N_HT):
                nc.sync.dma_start(out=w1bs[ih][:], in_=w1p[:, ih, :, :])
                nc.sync.dma_start(out=w3bs[ih][:], in_=w3p[:, ih, :, :])
            ht = ht_pool.tile([P, N_HT, T], BF16)

            # stage 1: HT[h, t] = silu(x @ w1.T).T * (x @ w3.T).T
            # token chunk its+1 is prefetched at the top of group its so the
            # group boundary never waits on DMA (xt pool double-buffers)
            xts = [xc0] + [None] * (T // TSUB - 1)
            for its in range(T // TSUB):
                ts0 = its * TSUB
                if its + 1 < T // TSUB:
                    nxt = xt_pool.tile([P, N_DN, TSUB], BF16, name="xc")
                    nc.scalar.dma_start(out=nxt[:], in_=xtp[:, its + 1, :, :])
                    xts[its + 1] = nxt
                xt_c = xts[its]
                for ih in range(N_HT):
                    ps1 = ps1_pool.tile([P, TSUB], F32)
                    ps2 = ps2_pool.tile([P, TSUB], F32)
                    # w1 and w3 chains interleaved per n-block: each x
                    # quarter is consumed by both chains as soon as it
                    # lands, halving the early DMA demand per PE-second
                    for n in range(N_DN):
                        nc.tensor.matmul(
                            ps1[:],
                            lhsT=w1bs[ih][:, n, :],
                            rhs=xt_c[:, n, :],
                            start=(n == 0),
                            stop=(n == N_DN - 1),
                        )
                        nc.tensor.matmul(
                            ps2[:],
                            lhsT=w3bs[ih][:, n, :],
                            rhs=xt_c[:, n, :],
                            start=(n == 0),
                            stop=(n == N_DN - 1),
                        )
                    tmp = tmp_pool.tile([P, TSUB], F32)
                    nc.scalar.activation(tmp[:], ps1[:], SILU)
                    nc.vector.tensor_mul(
                        ht[:, ih, ts0:ts0 + TSUB], tmp[:], ps2[:]
                    )

            # stage 2: OUT.T[d, t] = sum_h W2T[h, d] * HT[h, t]
            for idt in range(N_DT):
                w2b = w2_pool.tile([P, N_HT, P], BF16)
                nc.sync.dma_start(out=w2b[:], in_=w2p[:, idt, :, :])
                for its in range(T // TSUB):
                    ts0 = its * TSUB
                    last = (idt == N_DT - 1) and (its == T // TSUB - 1)
                    # the very last chunk runs as two half-width chains so
                    # the copy+store of the first half hides behind the
                    # second half's matmuls; the final store issues on the
                    # idle Sync queue, shortening the kernel tail
                    pieces = ([(0, 256, nc.scalar), (256, 256, nc.sync)]
                              if last else [(0, TSUB, nc.scalar)])
                    for h0, hw, eng in pieces:
                        pso = pso_pool.tile([P, TSUB], F32, name="pso")
                        for hn in range(N_HT):
                            nc.tensor.matmul(
                                pso[:, 0:hw],
                                lhsT=w2b[:, hn, :],
                                rhs=ht[:, hn, ts0 + h0:ts0 + h0 + hw],
                                start=(hn == 0),
                                stop=(hn == N_HT - 1),
                            )
                        ob = ob_pool.tile([P, TSUB], BF16, name="ob")
                        nc.vector.tensor_copy(ob[:, 0:hw], pso[:, 0:hw])
                        eng.dma_start(
                            out=outt[idt * P:(idt + 1) * P,
                                     ts0 + h0:ts0 + h0 + hw],
                            in_=ob[:, 0:hw],
                        )
    nc.compile()
    return nc


_CACHE: dict = {}


def _get_nc() -> bass.Bass:
    if "nc" not in _CACHE:
        _CACHE["nc"] = _build_program()
    return _CACHE["nc"]


def _pack_weights(w1, w2, w3):
    maps = []
    for e in range(N_EXPERTS):
        maps.append({
            "w1p": np.ascontiguousarray(
                w1[e].reshape(N_HT, P, N_DN, P).transpose(3, 0, 2, 1).astype(BF)),
            "w3p": np.ascontiguousarray(
                w3[e].reshape(N_HT, P, N_DN, P).transpose(3, 0, 2, 1).astype(BF)),
            "w2p": np.ascontiguousarray(
                w2[e].reshape(N_DT, P, N_HT, P).transpose(3, 0, 2, 1).astype(BF)),
        })
    return maps


def kernel(x, w1, w2, w3, num_tokens_per_expert, _trace=False):
    x = np.ascontiguousarray(np.asarray(x, dtype=np.float32))
    w1 = np.ascontiguousarray(np.asarray(w1, dtype=np.float32))
    w2 = np.ascontiguousarray(np.asarray(w2, dtype=np.float32))
    w3 = np.ascontiguousarray(np.asarray(w3, dtype=np.float32))
    counts = np.asarray(num_tokens_per_expert, dtype=np.int64)

    cs = np.cumsum(counts)
    starts = np.minimum(np.concatenate([[0], cs[:-1]]), N_TOKENS)
    ends = np.minimum(cs, N_TOKENS)
    lens = np.maximum(ends - starts, 0)

    wmaps = _pack_weights(w1, w2, w3)
    out = np.zeros((N_TOKENS, DIM), np.float32)
    trace_info = []

    n_passes = max(1, int(np.max(np.ceil(lens / T))))
    for k in range(n_passes):
        in_maps = []
        for e in range(N_EXPERTS):
            s = int(starts[e]) + k * T
            xe = np.zeros((T, DIM), np.float32)
            avail = x[s:s + T]
            if avail.shape[0]:
                xe[:avail.shape[0]] = avail
            # [P, n_chunks, N_DN, TSUB]: xtp[p, c, n, t] = x[c*TSUB+t, n*128+p]
            xtp = np.ascontiguousarray(
                xe.T.reshape(N_DN, P, T // TSUB, TSUB)
                .transpose(1, 2, 0, 3).astype(BF))
            in_maps.append({"xtp": xtp, **wmaps[e]})
        res = run_bass_kernel_spmd(
            _get_nc(), in_maps, list(range(N_CORES)), trace=_trace
        )
        if _trace:
            trace_info.append(res)
        for e in range(N_EXPERTS):
            s = int(starts[e]) + k * T
            cnt = min(int(ends[e]) - s, T)
            if cnt > 0:
                out[s:s + cnt] = res.results[e]["outt"].T[:cnt].astype(np.float32)

    if _trace:
        return out, trace_info
    return out
